# revision 30
# baseline (speedup 1.0000x reference)
"""BiDAF attention kernel for 8 Trainium2 NeuronCores (data-parallel over batch).

Contract: kernel(**inputs) takes the FULL unsharded inputs (as produced by the
reference setup_inputs) and returns the FULL [16, 1024, 2048] fp32 output.

Math (per batch b):
    s[i,j]  = c[i].c_w + q[j].q_w + sum_h c[i,h]*cqw[h]*q[j,h] + bias
    s1      = softmax_j(masked(s, q_mask));  s2 = softmax_i(masked(s, c_mask))
    a       = s1 @ q ; bb = s1 @ s2^T @ c
    out     = concat(c, a, c*a, c*bb)

Device mapping (per core: 2 batches), v3 = bf16 matmul pipeline:
  - Host folds cq_weight and c_weight into the q side (qw' = q*cqw + c_w) and
    ships bf16 copies of c in BOTH layouts: c[i,h] and cT[h,i].  That removes
    all on-device context transposes and their PSUM round-trips; the
    similarity chain reads cT straight from HBM.  sim_q + bias + q_mask fold
    into the Exp activation's per-partition fp32 bias.
  - All matmuls run in bf16 (error budget: rel tol is 2e-2, bf16 chain lands
    ~1e-3): sT = qw'T.T @ cT; exp -> e (bf16); a = (e@q)*r1; t = (eN@c)*r2;
    b = (e@t)*r1.
  - One exp serves both softmaxes (no max-subtraction; values bounded).
    r2 row-sums fall out of the Exp accum_out; r1 column-sums fall out of a
    single DVE 3D reduce over the e-transpose PSUM tile.
  - c*a runs alternately on DVE and GpSimd (both SBUF-only operands);
    c*b is one DVE scalar_tensor_tensor (pb * r1) * c straight out of PSUM.
  - Outputs are fp32; they stream out in 0.5-1 MB HWDGE stores as soon as
    each i-tile pair is ready.  The exact c block of the output is assembled
    host-side (pure copy of an input).
"""

import os
import sys
from contextlib import ExitStack

import numpy as np
import ml_dtypes

for _p in ("/opt/trn_rl_repo", "/root/.axon_site/_ro/trn_rl_repo"):
    if os.path.isdir(_p) and _p not in sys.path:
        sys.path.append(_p)

B, CL, QL, H = 16, 1024, 128, 512
N_CORES = 8
BPC = B // N_CORES  # batches per core
NEG = np.float32(-1e30)
BF = ml_dtypes.bfloat16

KT = H // 128  # 4 k-tiles over the hidden dim
IT = CL // 128  # 8 i-tiles over the context dim

_build_cache = {}


def _build(mask_trivial: bool):
    key = mask_trivial
    if key in _build_cache:
        return _build_cache[key]

    import concourse.bass as bass
    import concourse.tile as tile
    from concourse import bacc, mybir

    F32 = mybir.dt.float32
    BF16 = mybir.dt.bfloat16
    AF = mybir.ActivationFunctionType
    MUL = mybir.AluOpType.mult
    PSUM = bass.MemorySpace.PSUM

    nc = bacc.Bacc("TRN2", target_bir_lowering=False, debug=False)

    cbf_d = nc.dram_tensor("cbf", [BPC, CL, H], BF16, kind="ExternalInput")
    ctb_d = nc.dram_tensor("ctb", [BPC, H, CL], BF16, kind="ExternalInput")
    # qpk[:, 0:512] = qw'.T k-tiles (bf16), [:, 512:1024] = q (bf16)
    qpk_d = nc.dram_tensor("qpk", [BPC, 128, 1024], BF16, kind="ExternalInput")
    qb_d = nc.dram_tensor("qb", [128, BPC], F32, kind="ExternalInput")
    ident_d = nc.dram_tensor("ident", [128, 128], BF16, kind="ExternalInput")
    if not mask_trivial:
        cmask_d = nc.dram_tensor("cmaskb", [BPC, 1, CL], BF16, kind="ExternalInput")
        onesr_d = nc.dram_tensor("onesr", [1, QL], BF16, kind="ExternalInput")
    # Outputs stream out as bf16 (host upcasts to fp32): the math already
    # carries bf16-level error and the tolerance budget is 2e-2, so halving
    # the store traffic is free accuracy-wise.
    aca_d = nc.dram_tensor("out_aca", [BPC, CL, 2 * H], BF16, kind="ExternalOutput")
    cb_d = nc.dram_tensor("out_cb", [BPC, CL, H], BF16, kind="ExternalOutput")

    with tile.TileContext(nc) as tc, ExitStack() as ctx:
        const = ctx.enter_context(tc.tile_pool(name="const", bufs=1))
        sbp = ctx.enter_context(tc.tile_pool(name="sbp", bufs=2))
        outp = ctx.enter_context(tc.tile_pool(name="outp", bufs=2))
        ps_acc = ctx.enter_context(tc.tile_pool(name="ps_acc", bufs=2, space=PSUM))
        ps_tr = ctx.enter_context(tc.tile_pool(name="ps_tr", bufs=2, space=PSUM))
        ps_ab = ctx.enter_context(tc.tile_pool(name="ps_ab", bufs=4, space=PSUM))

        # ---- phase A: all HBM loads up front, HWDGE only; first the slices
        # the front end needs soonest.
        LD = []
        for bi in range(BPC):
            # k-major contiguous loads (512KB each at full HBM BW); the sim
            # accumulation chain can start after the first one lands.
            ctb = sbp.tile([128, KT, CL], BF16, tag="ct")
            for kh in range(2):
                src = ctb_d.ap()[bi, kh * 256 : (kh + 1) * 256, :].rearrange(
                    "(t p) i -> p t i", p=128
                )
                nc.sync.dma_start(ctb[:, kh * 2 : (kh + 1) * 2, :], src)
                if bi == 0 and kh == 0:
                    qpk = sbp.tile([128, 1024], BF16, tag="qp")
                    nc.sync.dma_start(qpk[:], qpk_d.ap()[bi])
                    qb = const.tile([128, BPC], F32, tag="qb")
                    nc.sync.dma_start(qb[:], qb_d.ap())
                    ident = const.tile([128, 128], BF16, tag="ident")
                    nc.sync.dma_start(ident[:], ident_d.ap())
            if bi > 0:
                qpk = sbp.tile([128, 1024], BF16, tag="qp")
                nc.sync.dma_start(qpk[:], qpk_d.ap()[bi])
            call = sbp.tile([128, IT, H], BF16, tag="call")
            nc.sync.dma_start(
                call[:], cbf_d.ap()[bi].rearrange("(t p) h -> p t h", p=128)
            )
            LD.append((ctb, call, qpk))
        if not mask_trivial:
            cmask_f = const.tile([1, BPC * CL], BF16, tag="cmask_f")
            nc.sync.dma_start(cmask_f[:], cmask_d.ap().rearrange("b one i -> one (b i)"))
            onesr_f = const.tile([1, QL], BF16, tag="onesr_f")
            nc.sync.dma_start(onesr_f[:], onesr_d.ap())

        # ---- PE clock warmup + ACT exp-table preload while loads stream.
        warmf = const.tile([128, 1], F32, tag="warmf")
        nc.vector.memset(warmf[:], 0.0)
        nc.scalar.activation(warmf[:, 0:1], warmf[:, 0:1], AF.Exp)
        warmL = const.tile([128, 1], BF16, tag="warmL")
        warmC = const.tile([128, 512], BF16, tag="warmC")
        nc.vector.memset(warmL[:], 0.0)
        nc.vector.memset(warmC[:], 0.0)
        pw = ps_acc.tile([QL, 512], F32, tag="acc")
        for _ in range(5):
            nc.tensor.matmul(pw[:1, :], warmL[:], warmC[:], start=True, stop=True)

        # ---- per-batch pipeline, software-pipelined across batches:
        #   F(b0) A(b0) F(b1) B2(b0) A(b1) B2(b1)
        # so batch-1's similarity front end runs while batch-0's b-path
        # drains, keeping the store stream gap-free.
        ST = {}

        def b1_front(bi):
            ctb, call, qpk = LD[bi]
            c_sb = [call[:, it, :] for it in range(IT)]
            qwT = qpk[:, 0:512].rearrange("p (t j) -> p t j", t=KT)
            qbias = qb[:, bi : bi + 1]
            rs2 = sbp.tile([QL, 2], F32, tag="rs2")
            ehalf, eNs, r1h = [], [], []
            for nh in range(2):
                spt = ps_acc.tile([QL, 512], F32, tag="acc")
                for k in range(KT):
                    nc.tensor.matmul(
                        spt[:],
                        qwT[:, k, :],
                        ctb[:, k, nh * 512 : (nh + 1) * 512],
                        start=(k == 0),
                        stop=(k == KT - 1 and mask_trivial),
                    )
                if not mask_trivial:
                    nc.tensor.matmul(
                        spt[:],
                        onesr_f[:],
                        cmask_f[:, bi * CL + nh * 512 : bi * CL + (nh + 1) * 512],
                        start=False,
                        stop=True,
                    )

                eh = sbp.tile([QL, 512], BF16, tag=f"e{nh}")
                nc.scalar.activation(
                    eh[:],
                    spt[:],
                    AF.Exp,
                    bias=qbias,
                    scale=1.0,
                    accum_out=rs2[:, nh : nh + 1],
                )
                ehalf.append(eh)

                # eN = e^T per j-block; r1 (s1 normalizers) via one DVE
                # 3D reduce over the transpose PSUM tile.
                pe = ps_tr.tile([128, 512], BF16, tag="tr")
                for j in range(4):
                    nc.tensor.transpose(
                        pe[:, j * 128 : (j + 1) * 128],
                        eh[:, j * 128 : (j + 1) * 128],
                        ident[:],
                    )
                csum = sbp.tile([128, 4], F32, tag=f"cs{nh}")
                nc.vector.tensor_reduce(
                    csum[:],
                    pe[:].rearrange("p (j q) -> p j q", j=4),
                    mybir.AxisListType.X,
                    mybir.AluOpType.add,
                )
                r1n = sbp.tile([128, 4], F32, tag=f"r1{nh}")
                nc.vector.reciprocal(r1n[:], csum[:])
                r1h.append(r1n)
                eNh = sbp.tile([128, 4, 128], BF16, tag=f"eN{nh}")
                nc.vector.tensor_copy(eNh[:], pe[:].rearrange("p (j q) -> p j q", j=4))
                eNs.append(eNh)
            ST[bi] = (c_sb, qpk, rs2, ehalf, eNs, r1h)

        def a_path(bi):
            c_sb, qpk, rs2, ehalf, eNs, r1h = ST[bi]
            q_sb = qpk[:, 512:1024]
            for nh in range(2):
                eh, r1n = ehalf[nh], r1h[nh]
                first = bi == 0 and nh == 0
                aca_sb = outp.tile([128, 4, 2 * H], BF16, tag="aca")
                for j in range(4):
                    it = 4 * nh + j
                    esl = eh[:, j * 128 : (j + 1) * 128]
                    pa = ps_ab.tile([128, H], F32, tag="ab")
                    nc.tensor.matmul(pa[:], esl, q_sb, start=True, stop=True)
                    nc.scalar.mul(aca_sb[:, j, 0:H], pa[:], r1n[:, j : j + 1])
                    # all-bf16 SBUF multiply hits the 2x DVE mode (~410ns);
                    # keeping GpSimd out of the a-path avoids the SBUF-port
                    # contention that was knocking DVE out of 2-port mode.
                    nc.vector.tensor_mul(
                        aca_sb[:, j, H : 2 * H], c_sb[it], aca_sb[:, j, 0:H]
                    )
                    if first:
                        rows = aca_d.ap()[
                            bi, nh * 512 + j * 128 : nh * 512 + (j + 1) * 128
                        ].rearrange("(t p) h -> p t h", p=128)
                        nc.sync.dma_start(rows[:], aca_sb[:, j : j + 1, :])
                    elif j % 2 == 1:
                        rows = aca_d.ap()[
                            bi, nh * 512 + (j - 1) * 128 : nh * 512 + (j + 1) * 128
                        ].rearrange("(t p) h -> p t h", p=128)
                        nc.sync.dma_start(rows[:], aca_sb[:, j - 1 : j + 1, :])

        def b_path(bi):
            c_sb, qpk, rs2, ehalf, eNs, r1h = ST[bi]
            rsum = sbp.tile([QL, 1], F32, tag="rsum")
            nc.vector.tensor_reduce(
                rsum[:], rs2[:], mybir.AxisListType.X, mybir.AluOpType.add
            )
            r2 = sbp.tile([QL, 1], F32, tag="r2")
            nc.vector.reciprocal(r2[:], rsum[:])

            ptraw = ps_acc.tile([QL, H], F32, tag="acc")
            for it in range(IT):
                nc.tensor.matmul(
                    ptraw[:],
                    eNs[it // 4][:, it % 4, :],
                    c_sb[it],
                    start=(it == 0),
                    stop=(it == IT - 1),
                )
            t_sb = sbp.tile([QL, H], BF16, tag="t")
            nc.scalar.mul(t_sb[:], ptraw[:], r2[:])

            for nh in range(2):
                # batch 0's b-path overlaps batch 1's front end where DVE is
                # the saturated engine and ACT/GpSimd have slack; same for
                # the final half's tail.  Spread those c*b muls off DVE.
                spread = bi == 0 or nh == 1
                cb_sb = outp.tile([128, 4, H], BF16, tag="cb")
                for j in range(4):
                    it = 4 * nh + j
                    esl = ehalf[nh][:, j * 128 : (j + 1) * 128]
                    pb = ps_ab.tile([128, H], F32, tag="ab")
                    nc.tensor.matmul(pb[:], esl, t_sb[:], start=True, stop=True)
                    if spread and j % 2 == 1:
                        btmp = sbp.tile([128, H], BF16, tag="btmp")
                        nc.scalar.mul(btmp[:], pb[:], r1h[nh][:, j : j + 1])
                        nc.gpsimd.tensor_mul(cb_sb[:, j, :], c_sb[it], btmp[:])
                    else:
                        nc.vector.scalar_tensor_tensor(
                            cb_sb[:, j, :],
                            pb[:],
                            r1h[nh][:, j : j + 1],
                            c_sb[it],
                            MUL,
                            MUL,
                        )
                    if j % 2 == 1:
                        rows = cb_d.ap()[
                            bi, nh * 512 + (j - 1) * 128 : nh * 512 + (j + 1) * 128
                        ].rearrange("(t p) h -> p t h", p=128)
                        nc.sync.dma_start(rows[:], cb_sb[:, j - 1 : j + 1, :])

        b1_front(0)
        a_path(0)
        b1_front(1)
        b_path(0)
        a_path(1)
        b_path(1)

    nc.compile()
    _build_cache[key] = nc
    return nc


def _install_profshim():
    """Optional NTFF profiling support (BIDAF_PROFILE=1); self-contained."""
    import contextlib
    import ctypes
    import types

    if "antenv.axon_hooks" in sys.modules:
        return
    so_path = "/opt/axon/libaxon_pjrt.so"
    try:
        lib = ctypes.CDLL(so_path)
    except OSError:
        return
    if not hasattr(lib, "axon_start_nrt_profile"):
        return
    lib.axon_start_nrt_profile.argtypes = [ctypes.POINTER(ctypes.c_int64), ctypes.c_size_t]
    lib.axon_start_nrt_profile.restype = ctypes.c_int64
    lib.axon_stop_nrt_profile.argtypes = [ctypes.c_char_p]
    lib.axon_stop_nrt_profile.restype = ctypes.c_int64

    @contextlib.contextmanager
    def _hook(output_dir, device_ids):
        import jax

        jax.devices()
        if device_ids:
            ids = (ctypes.c_int64 * len(device_ids))(*device_ids)
            rc = lib.axon_start_nrt_profile(ids, len(device_ids))
        else:
            rc = lib.axon_start_nrt_profile(None, 0)
        if rc != 0:
            raise RuntimeError(f"axon_start_nrt_profile rc={rc}")
        try:
            yield
        finally:
            n = lib.axon_stop_nrt_profile(str(output_dir).encode())
            print(f"profile: {n} file(s) written to {output_dir}")

    mod = types.ModuleType("antenv.axon_hooks")
    mod.get_axon_ntff_profile_hook = lambda: _hook
    mod.set_axon_ntff_profile_hook = lambda h: None
    sys.modules["antenv.axon_hooks"] = mod
    import antenv

    antenv.axon_hooks = mod

    from concourse import bass_utils

    bass_utils.upload_artifacts = lambda tmpdir: f"local:{tmpdir}"


def kernel(c, q, c_mask, q_mask, c_weight, q_weight, cq_weight, bias):
    from concourse.bass_utils import run_bass_kernel_spmd

    c = np.asarray(c, dtype=np.float32)
    q = np.asarray(q, dtype=np.float32)
    c_mask = np.asarray(c_mask)
    q_mask = np.asarray(q_mask)
    c_weight = np.asarray(c_weight, dtype=np.float32)
    q_weight = np.asarray(q_weight, dtype=np.float32)
    cq_weight = np.asarray(cq_weight, dtype=np.float32)
    bias = np.asarray(bias, dtype=np.float32)

    # host-side folding + bf16 input marshalling
    qw = q * cq_weight.reshape(1, 1, H) + c_weight.reshape(1, 1, H)  # [B, QL, H]
    sim_q = (q @ q_weight)[:, :, 0]  # [B, QL]
    amask_q = (1.0 - q_mask.astype(np.float32)) * NEG
    qbias = (sim_q + bias[0] + amask_q).astype(np.float32)  # [B, QL]
    amask_c = ((1.0 - c_mask.astype(np.float32)) * NEG).reshape(B, 1, CL)
    mask_trivial = bool((amask_c == 0).all())

    cbf = c.astype(BF)  # [B, CL, H]
    ctb = np.ascontiguousarray(cbf.transpose(0, 2, 1))  # [B, H, CL]
    qpk = np.empty((B, 128, 1024), dtype=BF)
    qpk[:, :, 0:512] = (
        qw.reshape(B, QL, KT, 128).transpose(0, 3, 2, 1).reshape(B, 128, KT * QL)
    ).astype(BF)
    qpk[:, :, 512:1024] = q.astype(BF)

    profile = os.environ.get("BIDAF_PROFILE", "") == "1"
    if profile:
        _install_profshim()

    nc = _build(mask_trivial)

    ident = np.eye(128, dtype=BF)
    in_maps = []
    for core in range(N_CORES):
        s = slice(BPC * core, BPC * (core + 1))
        m = {
            "cbf": np.ascontiguousarray(cbf[s]),
            "ctb": np.ascontiguousarray(ctb[s]),
            "qpk": np.ascontiguousarray(qpk[s]),
            "qb": np.ascontiguousarray(qbias[s].T),
            "ident": ident,
        }
        if not mask_trivial:
            m["cmaskb"] = np.ascontiguousarray(amask_c[s]).astype(BF)
            m["onesr"] = np.ones((1, QL), dtype=BF)
        in_maps.append(m)

    kw = {}
    if profile:
        kw = dict(trace=True, tmpdir=os.environ.get("BIDAF_PROFILE_DIR") or None)
    res = run_bass_kernel_spmd(nc, in_maps, list(range(N_CORES)), **kw)
    if profile and res.exec_time_ns is not None:
        print(f"[kernel] HW exec time: {res.exec_time_ns} ns")
        kernel.last_exec_time_ns = res.exec_time_ns
        kernel.last_trace = res.instructions_and_trace[1] if res.instructions_and_trace else None

    out = np.empty((B, CL, 4 * H), dtype=np.float32)
    out[:, :, 0:H] = c
    for i in range(N_CORES):
        out[BPC * i : BPC * (i + 1), :, H : 3 * H] = res.results[i]["out_aca"]
        out[BPC * i : BPC * (i + 1), :, 3 * H :] = res.results[i]["out_cb"]
    return out


kernel.last_exec_time_ns = None
kernel.last_trace = None


# revision 33
# speedup vs baseline: 1.0188x; 1.0188x over previous
"""BiDAF attention kernel for 8 Trainium2 NeuronCores (data-parallel over batch).

Contract: kernel(**inputs) takes the FULL unsharded inputs (as produced by the
reference setup_inputs) and returns the FULL [16, 1024, 2048] fp32 output.

Math (per batch b):
    s[i,j]  = c[i].c_w + q[j].q_w + sum_h c[i,h]*cqw[h]*q[j,h] + bias
    s1      = softmax_j(masked(s, q_mask));  s2 = softmax_i(masked(s, c_mask))
    a       = s1 @ q ; bb = s1 @ s2^T @ c
    out     = concat(c, a, c*a, c*bb)

Device mapping (per core: 2 batches), v3 = bf16 matmul pipeline:
  - Host folds cq_weight and c_weight into the q side (qw' = q*cqw + c_w) and
    ships bf16 copies of c in BOTH layouts: c[i,h] and cT[h,i].  That removes
    all on-device context transposes and their PSUM round-trips; the
    similarity chain reads cT straight from HBM.  sim_q + bias + q_mask fold
    into the Exp activation's per-partition fp32 bias.
  - All matmuls run in bf16 (error budget: rel tol is 2e-2, bf16 chain lands
    ~1e-3): sT = qw'T.T @ cT; exp -> e (bf16); a = (e@q)*r1; t = (eN@c)*r2;
    b = (e@t)*r1.
  - One exp serves both softmaxes (no max-subtraction; values bounded).
    r2 row-sums fall out of the Exp accum_out; r1 column-sums fall out of a
    single DVE 3D reduce over the e-transpose PSUM tile.
  - c*a runs alternately on DVE and GpSimd (both SBUF-only operands);
    c*b is one DVE scalar_tensor_tensor (pb * r1) * c straight out of PSUM.
  - Outputs are fp32; they stream out in 0.5-1 MB HWDGE stores as soon as
    each i-tile pair is ready.  The exact c block of the output is assembled
    host-side (pure copy of an input).
"""

import os
import sys
from contextlib import ExitStack

import numpy as np
import ml_dtypes

for _p in ("/opt/trn_rl_repo", "/root/.axon_site/_ro/trn_rl_repo"):
    if os.path.isdir(_p) and _p not in sys.path:
        sys.path.append(_p)

B, CL, QL, H = 16, 1024, 128, 512
N_CORES = 8
BPC = B // N_CORES  # batches per core
NEG = np.float32(-1e30)
BF = ml_dtypes.bfloat16

KT = H // 128  # 4 k-tiles over the hidden dim
IT = CL // 128  # 8 i-tiles over the context dim

_build_cache = {}


def _build(mask_trivial: bool):
    key = mask_trivial
    if key in _build_cache:
        return _build_cache[key]

    import concourse.bass as bass
    import concourse.tile as tile
    from concourse import bacc, mybir

    F32 = mybir.dt.float32
    BF16 = mybir.dt.bfloat16
    AF = mybir.ActivationFunctionType
    MUL = mybir.AluOpType.mult
    PSUM = bass.MemorySpace.PSUM

    nc = bacc.Bacc("TRN2", target_bir_lowering=False, debug=False)

    cbf_d = nc.dram_tensor("cbf", [BPC, CL, H], BF16, kind="ExternalInput")
    ctb_d = nc.dram_tensor("ctb", [BPC, H, CL], BF16, kind="ExternalInput")
    # qpk[:, 0:512] = qw'.T k-tiles (bf16), [:, 512:1024] = q (bf16)
    qpk_d = nc.dram_tensor("qpk", [BPC, 128, 1024], BF16, kind="ExternalInput")
    qb_d = nc.dram_tensor("qb", [128, BPC], F32, kind="ExternalInput")
    ident_d = nc.dram_tensor("ident", [128, 128], BF16, kind="ExternalInput")
    if not mask_trivial:
        cmask_d = nc.dram_tensor("cmaskb", [BPC, 1, CL], BF16, kind="ExternalInput")
        onesr_d = nc.dram_tensor("onesr", [1, QL], BF16, kind="ExternalInput")
    # Outputs stream out as bf16 (host upcasts to fp32): the math already
    # carries bf16-level error and the tolerance budget is 2e-2, so halving
    # the store traffic is free accuracy-wise.
    aca_d = nc.dram_tensor("out_aca", [BPC, CL, 2 * H], BF16, kind="ExternalOutput")
    cb_d = nc.dram_tensor("out_cb", [BPC, CL, H], BF16, kind="ExternalOutput")

    with tile.TileContext(nc) as tc, ExitStack() as ctx:
        const = ctx.enter_context(tc.tile_pool(name="const", bufs=1))
        sbp = ctx.enter_context(tc.tile_pool(name="sbp", bufs=2))
        outp = ctx.enter_context(tc.tile_pool(name="outp", bufs=2))
        ps_acc = ctx.enter_context(tc.tile_pool(name="ps_acc", bufs=2, space=PSUM))
        ps_tr = ctx.enter_context(tc.tile_pool(name="ps_tr", bufs=2, space=PSUM))
        ps_ab = ctx.enter_context(tc.tile_pool(name="ps_ab", bufs=4, space=PSUM))

        # ---- phase A: all HBM loads up front, HWDGE only; first the slices
        # the front end needs soonest.
        LD = []
        for bi in range(BPC):
            # k-major contiguous loads (512KB each at full HBM BW); the sim
            # accumulation chain can start after the first one lands.
            ctb = sbp.tile([128, KT, CL], BF16, tag="ct")
            for kh in range(2):
                src = ctb_d.ap()[bi, kh * 256 : (kh + 1) * 256, :].rearrange(
                    "(t p) i -> p t i", p=128
                )
                nc.sync.dma_start(ctb[:, kh * 2 : (kh + 1) * 2, :], src)
                if bi == 0 and kh == 0:
                    # qpk right after the first cT chunk: the sim chain's
                    # stationary (qw'T) and the k23 chunk must both land
                    # before qb/ident (needed later, at exp / eT).
                    qpk = sbp.tile([128, 1024], BF16, tag="qp")
                    nc.sync.dma_start(qpk[:], qpk_d.ap()[bi])
            if bi == 0:
                qb = const.tile([128, BPC], F32, tag="qb")
                nc.sync.dma_start(qb[:], qb_d.ap())
                ident = const.tile([128, 128], BF16, tag="ident")
                nc.sync.dma_start(ident[:], ident_d.ap())
            else:
                qpk = sbp.tile([128, 1024], BF16, tag="qp")
                nc.sync.dma_start(qpk[:], qpk_d.ap()[bi])
            call = sbp.tile([128, IT, H], BF16, tag="call")
            nc.sync.dma_start(
                call[:], cbf_d.ap()[bi].rearrange("(t p) h -> p t h", p=128)
            )
            LD.append((ctb, call, qpk))
        if not mask_trivial:
            cmask_f = const.tile([1, BPC * CL], BF16, tag="cmask_f")
            nc.sync.dma_start(cmask_f[:], cmask_d.ap().rearrange("b one i -> one (b i)"))
            onesr_f = const.tile([1, QL], BF16, tag="onesr_f")
            nc.sync.dma_start(onesr_f[:], onesr_d.ap())

        # ---- PE clock warmup + ACT exp-table preload while loads stream.
        warmf = const.tile([128, 1], F32, tag="warmf")
        nc.vector.memset(warmf[:], 0.0)
        nc.scalar.activation(warmf[:, 0:1], warmf[:, 0:1], AF.Exp)
        warmL = const.tile([128, 1], BF16, tag="warmL")
        warmC = const.tile([128, 512], BF16, tag="warmC")
        nc.vector.memset(warmL[:], 0.0)
        nc.vector.memset(warmC[:], 0.0)
        pw = ps_acc.tile([QL, 512], F32, tag="acc")
        for _ in range(6):
            nc.tensor.matmul(pw[:1, :], warmL[:], warmC[:], start=True, stop=True)

        # ---- per-batch pipeline, software-pipelined across batches:
        #   F(b0) A(b0) F(b1) B2(b0) A(b1) B2(b1)
        # so batch-1's similarity front end runs while batch-0's b-path
        # drains, keeping the store stream gap-free.
        ST = {}

        def b1_front(bi):
            ctb, call, qpk = LD[bi]
            c_sb = [call[:, it, :] for it in range(IT)]
            qwT = qpk[:, 0:512].rearrange("p (t j) -> p t j", t=KT)
            qbias = qb[:, bi : bi + 1]
            rs2 = sbp.tile([QL, 2], F32, tag="rs2")
            ehalf, eNs, r1h = [], [], []
            for nh in range(2):
                spt = ps_acc.tile([QL, 512], F32, tag="acc")
                for k in range(KT):
                    nc.tensor.matmul(
                        spt[:],
                        qwT[:, k, :],
                        ctb[:, k, nh * 512 : (nh + 1) * 512],
                        start=(k == 0),
                        stop=(k == KT - 1 and mask_trivial),
                    )
                if not mask_trivial:
                    nc.tensor.matmul(
                        spt[:],
                        onesr_f[:],
                        cmask_f[:, bi * CL + nh * 512 : bi * CL + (nh + 1) * 512],
                        start=False,
                        stop=True,
                    )

                eh = sbp.tile([QL, 512], BF16, tag=f"e{nh}")
                nc.scalar.activation(
                    eh[:],
                    spt[:],
                    AF.Exp,
                    bias=qbias,
                    scale=1.0,
                    accum_out=rs2[:, nh : nh + 1],
                )
                ehalf.append(eh)

                # eN = e^T per j-block; r1 (s1 normalizers) via one DVE
                # 3D reduce over the transpose PSUM tile.
                pe = ps_tr.tile([128, 512], BF16, tag="tr")
                for j in range(4):
                    nc.tensor.transpose(
                        pe[:, j * 128 : (j + 1) * 128],
                        eh[:, j * 128 : (j + 1) * 128],
                        ident[:],
                    )
                csum = sbp.tile([128, 4], F32, tag=f"cs{nh}")
                nc.vector.tensor_reduce(
                    csum[:],
                    pe[:].rearrange("p (j q) -> p j q", j=4),
                    mybir.AxisListType.X,
                    mybir.AluOpType.add,
                )
                r1n = sbp.tile([128, 4], F32, tag=f"r1{nh}")
                nc.vector.reciprocal(r1n[:], csum[:])
                r1h.append(r1n)
                eNh = sbp.tile([128, 4, 128], BF16, tag=f"eN{nh}")
                nc.vector.tensor_copy(eNh[:], pe[:].rearrange("p (j q) -> p j q", j=4))
                eNs.append(eNh)
            ST[bi] = (c_sb, qpk, rs2, ehalf, eNs, r1h)

        def a_path(bi):
            c_sb, qpk, rs2, ehalf, eNs, r1h = ST[bi]
            q_sb = qpk[:, 512:1024]
            for nh in range(2):
                eh, r1n = ehalf[nh], r1h[nh]
                first = bi == 0 and nh == 0
                aca_sb = outp.tile([128, 4, 2 * H], BF16, tag="aca")
                for j in range(4):
                    it = 4 * nh + j
                    esl = eh[:, j * 128 : (j + 1) * 128]
                    pa = ps_ab.tile([128, H], F32, tag="ab")
                    nc.tensor.matmul(pa[:], esl, q_sb, start=True, stop=True)
                    nc.scalar.mul(aca_sb[:, j, 0:H], pa[:], r1n[:, j : j + 1])
                    # all-bf16 SBUF multiply hits the 2x DVE mode (~410ns);
                    # keeping GpSimd out of the a-path avoids the SBUF-port
                    # contention that was knocking DVE out of 2-port mode.
                    nc.vector.tensor_mul(
                        aca_sb[:, j, H : 2 * H], c_sb[it], aca_sb[:, j, 0:H]
                    )
                    if first:
                        rows = aca_d.ap()[
                            bi, nh * 512 + j * 128 : nh * 512 + (j + 1) * 128
                        ].rearrange("(t p) h -> p t h", p=128)
                        nc.sync.dma_start(rows[:], aca_sb[:, j : j + 1, :])
                    elif j % 2 == 1:
                        rows = aca_d.ap()[
                            bi, nh * 512 + (j - 1) * 128 : nh * 512 + (j + 1) * 128
                        ].rearrange("(t p) h -> p t h", p=128)
                        nc.sync.dma_start(rows[:], aca_sb[:, j - 1 : j + 1, :])

        def b_path(bi):
            c_sb, qpk, rs2, ehalf, eNs, r1h = ST[bi]
            rsum = sbp.tile([QL, 1], F32, tag="rsum")
            nc.vector.tensor_reduce(
                rsum[:], rs2[:], mybir.AxisListType.X, mybir.AluOpType.add
            )
            r2 = sbp.tile([QL, 1], F32, tag="r2")
            nc.vector.reciprocal(r2[:], rsum[:])

            ptraw = ps_acc.tile([QL, H], F32, tag="acc")
            for it in range(IT):
                nc.tensor.matmul(
                    ptraw[:],
                    eNs[it // 4][:, it % 4, :],
                    c_sb[it],
                    start=(it == 0),
                    stop=(it == IT - 1),
                )
            t_sb = sbp.tile([QL, H], BF16, tag="t")
            nc.scalar.mul(t_sb[:], ptraw[:], r2[:])

            for nh in range(2):
                tail = bi == BPC - 1 and nh == 1
                cb_sb = outp.tile([128, 4, H], BF16, tag="cb")
                for j in range(4):
                    it = 4 * nh + j
                    esl = ehalf[nh][:, j * 128 : (j + 1) * 128]
                    pb = ps_ab.tile([128, H], F32, tag="ab")
                    nc.tensor.matmul(pb[:], esl, t_sb[:], start=True, stop=True)
                    if tail and j % 2 == 1:
                        # last half: spread the final c*b muls across ACT +
                        # GpSimd so the tail isn't serialized on DVE
                        btmp = sbp.tile([128, H], BF16, tag="btmp")
                        nc.scalar.mul(btmp[:], pb[:], r1h[nh][:, j : j + 1])
                        nc.gpsimd.tensor_mul(cb_sb[:, j, :], c_sb[it], btmp[:])
                    else:
                        nc.vector.scalar_tensor_tensor(
                            cb_sb[:, j, :],
                            pb[:],
                            r1h[nh][:, j : j + 1],
                            c_sb[it],
                            MUL,
                            MUL,
                        )
                    if j % 2 == 1:
                        rows = cb_d.ap()[
                            bi, nh * 512 + (j - 1) * 128 : nh * 512 + (j + 1) * 128
                        ].rearrange("(t p) h -> p t h", p=128)
                        nc.sync.dma_start(rows[:], cb_sb[:, j - 1 : j + 1, :])

        b1_front(0)
        a_path(0)
        b1_front(1)
        b_path(0)
        a_path(1)
        b_path(1)

    nc.compile()
    _build_cache[key] = nc
    return nc


def _install_profshim():
    """Optional NTFF profiling support (BIDAF_PROFILE=1); self-contained."""
    import contextlib
    import ctypes
    import types

    if "antenv.axon_hooks" in sys.modules:
        return
    so_path = "/opt/axon/libaxon_pjrt.so"
    try:
        lib = ctypes.CDLL(so_path)
    except OSError:
        return
    if not hasattr(lib, "axon_start_nrt_profile"):
        return
    lib.axon_start_nrt_profile.argtypes = [ctypes.POINTER(ctypes.c_int64), ctypes.c_size_t]
    lib.axon_start_nrt_profile.restype = ctypes.c_int64
    lib.axon_stop_nrt_profile.argtypes = [ctypes.c_char_p]
    lib.axon_stop_nrt_profile.restype = ctypes.c_int64

    @contextlib.contextmanager
    def _hook(output_dir, device_ids):
        import jax

        jax.devices()
        if device_ids:
            ids = (ctypes.c_int64 * len(device_ids))(*device_ids)
            rc = lib.axon_start_nrt_profile(ids, len(device_ids))
        else:
            rc = lib.axon_start_nrt_profile(None, 0)
        if rc != 0:
            raise RuntimeError(f"axon_start_nrt_profile rc={rc}")
        try:
            yield
        finally:
            n = lib.axon_stop_nrt_profile(str(output_dir).encode())
            print(f"profile: {n} file(s) written to {output_dir}")

    mod = types.ModuleType("antenv.axon_hooks")
    mod.get_axon_ntff_profile_hook = lambda: _hook
    mod.set_axon_ntff_profile_hook = lambda h: None
    sys.modules["antenv.axon_hooks"] = mod
    import antenv

    antenv.axon_hooks = mod

    from concourse import bass_utils

    bass_utils.upload_artifacts = lambda tmpdir: f"local:{tmpdir}"


def kernel(c, q, c_mask, q_mask, c_weight, q_weight, cq_weight, bias):
    from concourse.bass_utils import run_bass_kernel_spmd

    c = np.asarray(c, dtype=np.float32)
    q = np.asarray(q, dtype=np.float32)
    c_mask = np.asarray(c_mask)
    q_mask = np.asarray(q_mask)
    c_weight = np.asarray(c_weight, dtype=np.float32)
    q_weight = np.asarray(q_weight, dtype=np.float32)
    cq_weight = np.asarray(cq_weight, dtype=np.float32)
    bias = np.asarray(bias, dtype=np.float32)

    # host-side folding + bf16 input marshalling
    qw = q * cq_weight.reshape(1, 1, H) + c_weight.reshape(1, 1, H)  # [B, QL, H]
    sim_q = (q @ q_weight)[:, :, 0]  # [B, QL]
    amask_q = (1.0 - q_mask.astype(np.float32)) * NEG
    qbias = (sim_q + bias[0] + amask_q).astype(np.float32)  # [B, QL]
    amask_c = ((1.0 - c_mask.astype(np.float32)) * NEG).reshape(B, 1, CL)
    mask_trivial = bool((amask_c == 0).all())

    cbf = c.astype(BF)  # [B, CL, H]
    ctb = np.ascontiguousarray(cbf.transpose(0, 2, 1))  # [B, H, CL]
    qpk = np.empty((B, 128, 1024), dtype=BF)
    qpk[:, :, 0:512] = (
        qw.reshape(B, QL, KT, 128).transpose(0, 3, 2, 1).reshape(B, 128, KT * QL)
    ).astype(BF)
    qpk[:, :, 512:1024] = q.astype(BF)

    profile = os.environ.get("BIDAF_PROFILE", "") == "1"
    if profile:
        _install_profshim()

    nc = _build(mask_trivial)

    ident = np.eye(128, dtype=BF)
    in_maps = []
    for core in range(N_CORES):
        s = slice(BPC * core, BPC * (core + 1))
        m = {
            "cbf": np.ascontiguousarray(cbf[s]),
            "ctb": np.ascontiguousarray(ctb[s]),
            "qpk": np.ascontiguousarray(qpk[s]),
            "qb": np.ascontiguousarray(qbias[s].T),
            "ident": ident,
        }
        if not mask_trivial:
            m["cmaskb"] = np.ascontiguousarray(amask_c[s]).astype(BF)
            m["onesr"] = np.ones((1, QL), dtype=BF)
        in_maps.append(m)

    kw = {}
    if profile:
        kw = dict(trace=True, tmpdir=os.environ.get("BIDAF_PROFILE_DIR") or None)
    res = run_bass_kernel_spmd(nc, in_maps, list(range(N_CORES)), **kw)
    if profile and res.exec_time_ns is not None:
        print(f"[kernel] HW exec time: {res.exec_time_ns} ns")
        kernel.last_exec_time_ns = res.exec_time_ns
        kernel.last_trace = res.instructions_and_trace[1] if res.instructions_and_trace else None

    out = np.empty((B, CL, 4 * H), dtype=np.float32)
    out[:, :, 0:H] = c
    for i in range(N_CORES):
        out[BPC * i : BPC * (i + 1), :, H : 3 * H] = res.results[i]["out_aca"]
        out[BPC * i : BPC * (i + 1), :, 3 * H :] = res.results[i]["out_cb"]
    return out


kernel.last_exec_time_ns = None
kernel.last_trace = None
